# revision 47
# baseline (speedup 1.0000x reference)
"""Trainium2 Bass kernel for nn_GAT_attention_multi (gnn_message_passing).

Math (per batch, N=200, D=64):
  emb = LN(embeddings); uid=emb[0], iid=emb[1], ua = uid*emb[2:]   [N,D]
  value[i,j,:] = LN(ua_i*ua_j) collapses via Gram matrices:
      mu  = (UA UA^T)/D ; var = (UA2 UA2^T)/D - mu^2 ; r = rsqrt(var+eps)
  per head: scores_ij = ua_i.vq + ua_j.vk + (cq+ck+si+ab)  (rank-2, vq=W^T aq)
      alphas = softmax_j(leaky(scores)); c = alphas*r
  out[j,d] = 0.5*sum_h [ lnw_d*(ua_jd*S1_h[j,d] - S2_h[j]) + lnb_d*S3_h[j] ]
      S1 = sum_i c_ij ua_id ; S2 = sum_i (c*mu)_ij ; S3 = sum_i alphas_ij
  row0 = uid*iid ; final leaky_relu.

Layout: feature dim d on partitions 0:64, "ones/bias" index 64. rsqrt via
exp(-0.5*ln(x)) so all ACT funcs ({Exp,Ln,Copy,Abs}) share one table.
Emission is STAGE-major across the 4 local batches so the in-order engine
queues overlap batches. Batch data-parallel: 32 batches -> 8 cores x 4.
"""

import numpy as np

import concourse.bass as bass
import concourse.tile as tile
from concourse.tile import add_dep_helper
from concourse import bacc, mybir
from concourse.masks import make_identity
from concourse.bass_utils import run_bass_kernel_spmd

AF = mybir.ActivationFunctionType
ALU = mybir.AluOpType
F32 = mybir.dt.float32

B, NODES, D = 32, 202, 64
N = NODES - 2            # 200
NCORES = 8
BL = B // NCORES         # 4 batches per core
NP = 256                 # padded N (matmul moving dim)
EPS = 1e-5
CH = [(0, 128), (128, N - 128)]  # token chunks (start, count)
SLOPE = 0.01


def build_nc():
    nc = bacc.Bacc("TRN2", target_bir_lowering=False)

    emb = nc.dram_tensor("emb", [BL, NODES, D], F32, kind="ExternalInput")
    ln_w = nc.dram_tensor("ln_w", [D], F32, kind="ExternalInput")
    ln_b = nc.dram_tensor("ln_b", [D], F32, kind="ExternalInput")
    Ws = [
        (
            nc.dram_tensor(f"W{h}_w", [D, D], F32, kind="ExternalInput"),
            nc.dram_tensor(f"W{h}_b", [D], F32, kind="ExternalInput"),
            nc.dram_tensor(f"a{h}_w", [3 * D], F32, kind="ExternalInput"),
            nc.dram_tensor(f"a{h}_b", [1], F32, kind="ExternalInput"),
        )
        for h in (1, 2)
    ]
    out = nc.dram_tensor("out", [BL, N + 1, D], F32, kind="ExternalOutput")

    with tile.TileContext(nc) as tc:
        with (
            tc.tile_pool(name="consts", bufs=1) as consts,
            tc.tile_pool(name="work", bufs=5) as work,
            tc.tile_pool(name="scr", bufs=4) as scr,
            tc.tile_pool(name="ps_tr", bufs=1, space="PSUM") as ps_tr,
            tc.tile_pool(name="ps_gh", bufs=2, space="PSUM") as ps_gh,
            tc.tile_pool(name="ps_sc", bufs=3, space="PSUM") as ps_sc,
            tc.tile_pool(name="ps_rows", bufs=2, space="PSUM") as ps_rows,
        ):
            # ---- stage A: prefetch all inputs (2 strided DMAs) ----
            eA = consts.tile([128, BL, 64], F32, tag="eA")
            nc.sync.dma_start(
                out=eA, in_=emb[:, 0:128, :].rearrange("b p d -> p b d")
            )
            eB = consts.tile([NODES - 128, BL, 64], F32, tag="eB")
            nc.sync.dma_start(
                out=eB, in_=emb[:, 128:NODES, :].rearrange("b p d -> p b d")
            )

            # ---- constants (issue spread: SP light / ACT heavy / Pool small) ----
            ident = consts.tile([128, 128], F32)
            make_identity(nc, ident)
            ones_col = consts.tile([128, 1], F32)
            nc.vector.memset(ones_col, 1.0)
            eps_t = consts.tile([128, 1], F32)
            nc.vector.memset(eps_t, EPS)
            ones2d = consts.tile([64, N], F32)
            nc.vector.memset(ones2d, 1.0)

            lnwcE = consts.tile([64, 1], F32)
            nc.sync.dma_start(out=lnwcE, in_=ln_w[:, None])
            lnbcE = consts.tile([64, 1], F32)
            nc.sync.dma_start(out=lnbcE, in_=ln_b[:, None])
            lnw_half_col = consts.tile([64, 1], F32)
            nc.scalar.mul(out=lnw_half_col, in_=lnwcE, mul=0.5)
            lnb_half_row = consts.tile([1, 65], F32)
            nc.vector.memset(lnb_half_row[0:1, 64:65], 0.0)
            nc.sync.dma_start(out=lnb_half_row[0:1, 0:64], in_=ln_b[None, :])
            lnw_row = consts.tile([1, 65], F32)
            nc.vector.memset(lnw_row[0:1, 64:65], 0.0)
            nc.sync.dma_start(out=lnw_row[0:1, 0:64], in_=ln_w[None, :])

            vqks, abts = [], []
            for h, (W_w, W_b, a_w, a_b) in enumerate(Ws):
                Wext = consts.tile([64, 65], F32, tag=f"wext{h}")
                nc.scalar.dma_start(out=Wext[:, 0:64], in_=W_w[:, :])
                nc.scalar.dma_start(out=Wext[:, 64:65], in_=W_b[:, None])
                aqk = consts.tile([64, 3], F32, tag=f"aqk{h}")
                nc.gpsimd.dma_start(
                    out=aqk, in_=a_w.rearrange("(c d) -> d c", d=64)
                )
                vqk_ps = ps_tr.tile([65, 3], F32, tag="tr")
                nc.tensor.matmul(vqk_ps, Wext, aqk, start=True, stop=True)
                vqk = consts.tile([65, 3], F32, tag=f"vqk{h}")
                nc.scalar.copy(out=vqk, in_=vqk_ps)
                vqks.append(vqk)
                abt = consts.tile([65, 1], F32, tag=f"abt{h}")
                nc.gpsimd.dma_start(out=abt[64:65, 0:1], in_=a_b[None, :])
                # fold the query-side bias cq = W_b . aq into the score const
                nc.vector.tensor_add(
                    out=abt[64:65, 0:1], in0=abt[64:65, 0:1], in1=vqk[64:65, 0:1]
                )
                abts.append(abt)

            # ---- stage B+C: layernorm, transpose, ua (per batch) ----
            elns = []
            embTs, uats, ua2ts, ua_exts = [], [], [], []
            for b in range(BL):
                eln = work.tile([128, 2, 64], F32, tag="eln")
                for c, pcnt in ((0, 128), (1, NODES - 128)):
                    src_ap = (eA if c == 0 else eB)[:pcnt, b, :]
                    st = scr.tile([128, 6], F32, tag="bnst")
                    nc.vector.bn_stats(out=st[:pcnt], in_=src_ap)
                    mv = scr.tile([128, 2], F32, tag="bnmv")
                    nc.vector.bn_aggr(out=mv[:pcnt], in_=st[:pcnt])
                    sq = scr.tile([128, 1], F32, tag="lnsq")
                    nc.scalar.activation(
                        out=sq[:pcnt], in_=mv[:pcnt, 1:2], func=AF.Sqrt,
                        bias=eps_t[:pcnt],
                    )
                    rstd = scr.tile([128, 1], F32, tag="rstd")
                    nc.vector.reciprocal_approx_fast(out=rstd[:pcnt], in_=sq[:pcnt])
                    nc.vector.tensor_scalar(
                        out=eln[:pcnt, c, :],
                        in0=src_ap,
                        scalar1=mv[:pcnt, 0:1],
                        scalar2=rstd[:pcnt],
                        op0=ALU.subtract,
                        op1=ALU.mult,
                    )
                elns.append(eln)

                # ---- stage C (merged): transpose, gamma/beta, ua, ua^2 ----
                etr = ps_tr.tile([64, NODES], F32, tag="tr")
                nc.tensor.transpose(etr[:, 0:128], elns[b][:, 0, :], ident)
                nc.tensor.transpose(
                    etr[:, 128:NODES],
                    elns[b][: NODES - 128, 1, :],
                    ident[: NODES - 128, : NODES - 128],
                )
                embT = work.tile([64, NODES], F32, tag="embT")
                nc.vector.tensor_scalar(
                    out=embT, in0=etr, scalar1=lnwcE, scalar2=lnbcE,
                    op0=ALU.mult, op1=ALU.add,
                )
                uat = work.tile([65, NP], F32, tag="uat")
                nc.gpsimd.memset(uat[0:64, N:NP], 0.0)
                nc.gpsimd.memset(uat[64:65, :], 1.0)
                nc.vector.tensor_scalar_mul(
                    out=uat[0:64, 0:N], in0=embT[:, 2:NODES], scalar1=embT[:, 0:1]
                )
                ua2t = work.tile([64, NP], F32, tag="ua2t")
                nc.gpsimd.tensor_mul(out=ua2t, in0=uat[0:64, :], in1=uat[0:64, :])
                uae = []
                for ic, (t0, cnt) in enumerate(CH):
                    utr = ps_tr.tile([128, 65], F32, tag="tr")
                    nc.tensor.transpose(
                        utr[:cnt], uat[:, t0 : t0 + cnt], ident[:65, :65]
                    )
                    u = work.tile([128, 65], F32, tag=f"uae{ic}")
                    nc.scalar.copy(out=u[:cnt], in_=utr[:cnt])
                    uae.append(u)
                embTs.append(embT)
                uats.append(uat)
                ua2ts.append(ua2t)
                ua_exts.append(uae)

            # ---- stage D+E: Gram matrices, mu/rstd maps, score rows ----
            muss, rss, Rs = [], [], []
            for b in range(BL):
                mus, rs = [], []
                for ic, (t0, cnt) in enumerate(CH):
                    gh = ps_gh.tile([128, 512], F32, tag="gh")
                    nc.tensor.matmul(
                        gh[:cnt, 0:NP],
                        uats[b][0:64, t0 : t0 + cnt],
                        uats[b][0:64, :],
                        start=True, stop=True,
                    )
                    nc.tensor.matmul(
                        gh[:cnt, NP:512],
                        ua2ts[b][:, t0 : t0 + cnt],
                        ua2ts[b],
                        start=True, stop=True,
                    )
                    mu = work.tile([128, N], F32, tag=f"mu{ic}")
                    # mu tile holds -mu (sign folded); msq=(-mu)^2 is unchanged
                    nc.scalar.mul(out=mu[:cnt], in_=gh[:cnt, 0:N], mul=-1.0 / D)
                    msq = scr.tile([128, N], F32, tag=f"msq{ic}")
                    nc.gpsimd.tensor_mul(out=msq[:cnt], in0=mu[:cnt], in1=mu[:cnt])
                    var = scr.tile([128, N], F32, tag=f"var{ic}")
                    nc.vector.scalar_tensor_tensor(
                        out=var[:cnt], in0=gh[:cnt, NP : NP + N], scalar=1.0 / D,
                        in1=msq[:cnt], op0=ALU.mult, op1=ALU.subtract,
                    )
                    sdev = scr.tile([128, N], F32, tag=f"sdev{ic}")
                    last_sqrt = nc.scalar.activation(
                        out=sdev[:cnt], in_=var[:cnt], func=AF.Sqrt,
                        bias=eps_t[:cnt],
                    )
                    r_ = work.tile([128, N], F32, tag=f"r{ic}")
                    nc.vector.reciprocal_approx_fast(out=r_[:cnt], in_=sdev[:cnt])
                    mus.append(mu)
                    rs.append(r_)
                muss.append(mus)
                rss.append(rs)

                # ---- stage E (merged): score row pieces (sk, si) and R ----
                rp = ps_rows.tile([65, 512], F32, tag="rows")
                for h in range(2):
                    nc.tensor.matmul(
                        rp[64:65, h * NP : h * NP + NP],
                        vqks[h][:, 1:2],
                        uats[b],
                        start=True, stop=True,
                    )
                iidt = scr.tile([65, 1], F32, tag="iidt")
                nc.gpsimd.memset(iidt[64:65], 1.0)
                nc.vector.tensor_copy(out=iidt[0:64], in_=embTs[b][:, 1:2])
                vi2 = scr.tile([65, 2], F32, tag="vi2")
                for h in range(2):
                    nc.vector.tensor_mul(
                        out=vi2[:, h : h + 1], in0=iidt, in1=vqks[h][:, 2:3]
                    )
                nc.tensor.matmul(
                    rp[64:65, 504:506], ones_col[0:65], vi2, start=True, stop=True
                )
                R = work.tile([65, 2 * N], F32, tag="R")
                for h in range(2):
                    nc.gpsimd.tensor_scalar_mul(
                        out=R[0:64, h * N : h * N + N],
                        in0=ones2d,
                        scalar1=vqks[h][0:64, 0:1],
                    )
                    nc.vector.tensor_scalar(
                        out=R[64:65, h * N : h * N + N],
                        in0=rp[64:65, h * NP : h * NP + N],
                        scalar1=rp[64:65, 504 + h : 505 + h],
                        scalar2=abts[h][64:65, 0:1],
                        op0=ALU.add, op1=ALU.add,
                    )
                Rs.append(R)

            # ---- stage F: scores -> leaky -> exp -> eh, c, -mu*c ----
            ehss, css, cmpss = [], [], []
            for b in range(BL):
                ehs, cs, cmps = [], [], []
                for ic, (t0, cnt) in enumerate(CH):
                    sc = ps_sc.tile([128, 2 * N], F32, tag="sc")
                    nc.tensor.matmul(
                        sc[:cnt], uats[b][:, t0 : t0 + cnt], Rs[b],
                        start=True, stop=True,
                    )
                    # leaky(x) = 0.505x + 0.495|x|  (one PSUM operand per op)
                    sabs = scr.tile([128, 2 * N], F32, tag=f"sabs{ic}")
                    nc.scalar.activation(
                        out=sabs[:cnt], in_=sc[:cnt], func=AF.Abs,
                        scale=(1.0 - SLOPE) / 2.0,
                    )
                    lr = scr.tile([128, 2 * N], F32, tag=f"lr{ic}")
                    nc.vector.scalar_tensor_tensor(
                        out=lr[:cnt], in0=sc[:cnt], scalar=(1.0 + SLOPE) / 2.0,
                        in1=sabs[:cnt], op0=ALU.mult, op1=ALU.add,
                    )
                    e = work.tile([128, 2, N], F32, tag=f"e{ic}")
                    ssum = scr.tile([128, 2], F32, tag=f"ssum{ic}")
                    for h in range(2):
                        exp_inst = nc.scalar.activation(
                            out=e[:cnt, h, :], in_=lr[:cnt, h * N : h * N + N],
                            func=AF.Exp, accum_out=ssum[:cnt, h : h + 1],
                        )
                        if b == 0 and ic == 0 and h == 0:
                            # keep all stage-D Sqrts before any Exp so the ACT
                            # func table switches exactly once
                            add_dep_helper(exp_inst.ins, last_sqrt.ins, sync=False,
                                           reason="act-table ordering")
                    rinv = scr.tile([128, 2], F32, tag=f"rinv{ic}")
                    nc.vector.reciprocal_approx_fast(out=rinv[:cnt], in_=ssum[:cnt])
                    eh = work.tile([128, 2, N], F32, tag=f"eh{ic}")
                    for h in range(2):
                        nc.scalar.mul(
                            out=eh[:cnt, h, :], in_=e[:cnt, h, :],
                            mul=rinv[:cnt, h : h + 1],
                        )
                    rrep = bass.AP(
                        tensor=rss[b][ic].tensor, offset=rss[b][ic].offset,
                        ap=[rss[b][ic].ap[0], [0, 2], [1, N]],
                    )
                    c_ = work.tile([128, 2, N], F32, tag=f"c{ic}")
                    nc.gpsimd.tensor_mul(out=c_[:cnt], in0=eh[:cnt], in1=rrep[:cnt])
                    murep = bass.AP(
                        tensor=muss[b][ic].tensor, offset=muss[b][ic].offset,
                        ap=[muss[b][ic].ap[0], [0, 2], [1, N]],
                    )
                    cmp_ = work.tile([128, 2, N], F32, tag=f"cmp{ic}")
                    nc.gpsimd.tensor_mul(
                        out=cmp_[:cnt], in0=murep[:cnt], in1=c_[:cnt]
                    )
                    ehs.append(eh)
                    cs.append(c_)
                    cmps.append(cmp_)
                ehss.append(ehs)
                css.append(cs)
                cmpss.append(cmps)

            # ---- stage G+H per batch: sums, S1^T, correction, finals ----
            osb0 = consts.tile([128, BL, 64], F32, tag="osb0")
            osb1 = consts.tile([N + 1 - 128, BL, 64], F32, tag="osb1")
            for b in range(BL):
                s3ps = ps_sc.tile([1, 2 * N], F32, tag="sc")
                for ic, (t0, cnt) in enumerate(CH):
                    nc.tensor.matmul(
                        s3ps,
                        ones_col[:cnt],
                        ehss[b][ic][:cnt].rearrange("p h j -> p (h j)"),
                        start=(ic == 0), stop=(ic == 1),
                    )
                s2ps = ps_sc.tile([1, 2 * N], F32, tag="sc")
                for ic, (t0, cnt) in enumerate(CH):
                    nc.tensor.matmul(
                        s2ps,
                        ones_col[:cnt],
                        cmpss[b][ic][:cnt].rearrange("p h j -> p (h j)"),
                        start=(ic == 0), stop=(ic == 1),
                    )
                s3_sb = scr.tile([1, 2 * N], F32, tag="s3sb")
                nc.scalar.mul(out=s3_sb, in_=s3ps, mul=0.5)
                s2_sb = scr.tile([1, 2 * N], F32, tag="s2sb")
                nc.scalar.mul(out=s2_sb, in_=s2ps, mul=0.5)

                s1t = ps_rows.tile([65, 2 * N], F32, tag="rows")
                for ic, (t0, cnt) in enumerate(CH):
                    nc.tensor.matmul(
                        s1t,
                        ua_exts[b][ic][:cnt],
                        css[b][ic][:cnt].rearrange("p h j -> p (h j)"),
                        start=(ic == 0), stop=(ic == 1),
                    )
                corr = ps_rows.tile([65, 2 * N], F32, tag="rows")
                nc.tensor.matmul(corr, lnb_half_row, s3_sb, start=True, stop=False)
                nc.tensor.matmul(corr, lnw_row, s2_sb, start=False, stop=True)

                uarep = bass.AP(
                    tensor=uats[b].tensor, offset=uats[b].offset,
                    ap=[uats[b].ap[0], [0, 2], [1, N]],
                )
                tp = scr.tile([64, 2, N], F32, tag="tp")
                nc.vector.tensor_mul(
                    out=tp, in0=s1t[0:64].rearrange("p (h j) -> p h j", h=2),
                    in1=uarep[0:64],
                )
                # 0.5*lnw applies only to the S1 term; corr already carries it
                tpc = scr.tile([64, 2, N], F32, tag="tpc")
                nc.vector.scalar_tensor_tensor(
                    out=tpc, in0=tp, scalar=lnw_half_col,
                    in1=corr[0:64].rearrange("p (h j) -> p h j", h=2),
                    op0=ALU.mult, op1=ALU.add,
                )
                outT = scr.tile([64, N + 1], F32, tag="outT")
                hs = scr.tile([64, N], F32, tag="hs")
                nc.gpsimd.tensor_add(out=hs, in0=tpc[:, 0, :], in1=tpc[:, 1, :])
                nc.vector.scalar_tensor_tensor(
                    out=outT[:, 1 : N + 1], in0=hs, scalar=SLOPE, in1=hs,
                    op0=ALU.mult, op1=ALU.max,
                )
                uii = scr.tile([64, 1], F32, tag="uii")
                nc.vector.tensor_mul(
                    out=uii, in0=embTs[b][:, 0:1], in1=embTs[b][:, 1:2]
                )
                nc.vector.scalar_tensor_tensor(
                    out=outT[:, 0:1], in0=uii, scalar=SLOPE, in1=uii,
                    op0=ALU.mult, op1=ALU.max,
                )
                for ic, (o0, ocnt) in enumerate(((0, 128), (128, N + 1 - 128))):
                    otr = ps_tr.tile([128, 64], F32, tag="tr")
                    nc.tensor.transpose(
                        otr[:ocnt], outT[:, o0 : o0 + ocnt], ident[:64, :64]
                    )
                    dst = osb0 if ic == 0 else osb1
                    nc.scalar.copy(out=dst[:ocnt, b, :], in_=otr[:ocnt])

            for b0_, b1_ in ((0, 2), (2, 4)):
                nc.sync.dma_start(
                    out=out[b0_:b1_, 0:128, :].rearrange("b p d -> p b d"),
                    in_=osb0[:, b0_:b1_, :],
                )
                nc.gpsimd.dma_start(
                    out=out[b0_:b1_, 128 : N + 1, :].rearrange("b p d -> p b d"),
                    in_=osb1[:, b0_:b1_, :],
                )

    nc.compile()
    return nc


_NC = None


def _get_nc():
    global _NC
    if _NC is None:
        _NC = build_nc()
    return _NC


def kernel(**inputs) -> np.ndarray:
    nc = _get_nc()
    emb = np.ascontiguousarray(np.asarray(inputs["embeddings"], dtype=np.float32))
    shared = {
        k: np.ascontiguousarray(np.asarray(inputs[k], np.float32))
        for k in ("ln_w", "ln_b", "W1_w", "W1_b", "a1_w", "a1_b",
                  "W2_w", "W2_b", "a2_w", "a2_b")
    }
    in_maps = [
        {"emb": np.ascontiguousarray(emb[c * BL : (c + 1) * BL]), **shared}
        for c in range(NCORES)
    ]
    res = run_bass_kernel_spmd(nc, in_maps, core_ids=list(range(NCORES)))
    return np.concatenate([res.results[c]["out"] for c in range(NCORES)], axis=0)
